# revision 2
# baseline (speedup 1.0000x reference)
"""PointTransformerLayer Bass kernel for Trainium2 (8 NeuronCores).

Sharding: core c handles batch b = c//2, query half qh = c%2 (2048 queries),
against all N=4096 candidates of that batch.  Each core uploads its full
candidate set directly (x as fp8 [64,N], pos as f32 [3,N]) plus its query
half of pos — no on-device collectives, so cores run fully decoupled and
the program is core-invariant with the per-core difference carried entirely
by the uploaded data.

Query-side x is never needed (mean-q cancels in the softmax), so queries are
exactly the uploaded pos half.  Weights and selector constants are embedded
in the NEFF via inline_tensor.  The device returns the fp8 attention delta;
the host adds the f32 residual.

Device pipeline per core:
  - dist[q,n] = <pos_q, pos_n> - 0.5*||pos_n||^2 on PE (K=4 matmul with a
    folded -xx/2 row).  Per-row this is a positive-affine transform of the
    reference's -||pos_q - pos_n||^2, so top-k selection is identical.
  - exact top-16 per row on DVE: max8 / max_index / match_replace, 2 rounds.
  - per-(q,j) payload gather via SWDGE dma_gather (transpose mode) of bf16
    lossless-split rows [XV1 XV2 | wn1 wn2 | mk1 mk2] -> feature-major SBUF.
  - pair MLP (position encoding) and logits via PE matmuls; the bf16 halves
    are re-summed exactly inside the matmuls (fp32 PSUM accum).
  - softmax over j and the j-aggregation stay in (q,j)-column space: DVE
    tensor_reduce(axis=X) gives the per-query sums, a tiny PE matmul
    replicates attn across the 16 per-head features, and a second
    tensor_reduce produces agg^T directly — no PE transposes.
  - deltaT = Wfc @ agg^T + bias on PE; host re-transposes and adds x.
"""

import contextlib
import os
import sys

import numpy as np

for _p in ("/opt/trn_rl_repo", "/root/.axon_site/_ro/trn_rl_repo"):
    if os.path.isdir(_p) and _p not in sys.path:
        sys.path.insert(0, _p)

import jax

jax.config.update("jax_compilation_cache_dir", "/tmp/jax_comp_cache")
jax.config.update("jax_persistent_cache_min_entry_size_bytes", -1)
jax.config.update("jax_persistent_cache_min_compile_time_secs", 0.0)

import concourse.bass as bass
import concourse.bacc as bacc
import concourse.tile as tile
from concourse import mybir

B, N, CIN, COUT, K, H = 4, 4096, 64, 64, 16, 4
Q = N // 2            # queries per core
NT = Q // 128         # q-tiles per core
NCHUNK = N // 512     # dist matmul chunks
ROWU = 384            # bf16 units per gather row (768 bytes)
F32 = mybir.dt.float32
BF16 = mybir.dt.bfloat16
F8E4 = mybir.dt.float8e4
I16 = mybir.dt.int16
U32 = mybir.dt.uint32
AF = mybir.ActivationFunctionType
OP = mybir.AluOpType
AX = mybir.AxisListType

NEG_BIG = -1.0e30


def _ap(base, dims):
    """AP with explicit free dims (list of [stride, num]) over a tile slice."""
    return bass.AP(tensor=base.tensor, offset=base.offset, ap=[base.ap[0]] + dims)


def _consts():
    I128 = np.eye(128, dtype=np.float32)
    REPJ = np.zeros((16, 128), np.float32)
    for p in range(128):
        REPJ[p % 16, p] = 1.0
    I64 = np.eye(64, dtype=np.float32)
    II64 = np.concatenate([I64, I64], 0)
    NMK = np.zeros((128, H), np.float32)
    NMK[0:4, 0:4] = -np.eye(4)
    NMK[64:68, 0:4] = -np.eye(4)
    S16 = np.zeros((COUT, H), np.float32)
    for co in range(COUT):
        S16[co, co // 16] = 1.0 / 16.0
    RH4 = np.zeros((H, COUT), np.float32)
    for co in range(COUT):
        RH4[co // 16, co] = 1.0
    return I128, REPJ, II64, NMK, S16, RH4


def build_nc(inputs):
    import ml_dtypes

    bf16 = lambda a: np.asarray(a, np.float32).astype(ml_dtypes.bfloat16)
    w = {k: np.asarray(v, np.float32) for k, v in inputs.items()
         if k not in ("x", "pos")}
    I128c, REPJc, II64c, NMKc, S16c, RH4c = _consts()

    nc = bacc.Bacc(num_devices=8)

    x8T = nc.declare_dram_parameter("x8T", [CIN, N], F8E4, False)
    pqT = nc.declare_dram_parameter("pqT", [3, Q], F32, False)
    pnT = nc.declare_dram_parameter("pnT", [3, N], F32, False)
    dT = nc.declare_dram_parameter("dT", [COUT, Q], F8E4, True)

    ct = lambda name, a: nc.inline_tensor(np.ascontiguousarray(a), name=name)
    Wp1T = ct("Wp1T", w["Wp1"].T)
    bp1 = ct("bp1", w["bp1"][None, :])
    WvT65 = ct("WvT65", np.concatenate([w["Wv"].T, w["bv"][None, :]], 0))
    Wk = ct("Wk", w["Wk"])
    bkc = ct("bkc", w["bk"][:, None])
    Wp2 = ct("Wp2", w["Wp2"])
    Wp2T = ct("Wp2T", w["Wp2"].T)
    WfcT = ct("WfcT", w["Wfc"].T)
    bp2c = ct("bp2c", w["bp2"][:, None])
    bfc = ct("bfc", w["bfc"][None, :])
    S16 = ct("S16", S16c)
    I128 = ct("I128", I128c)
    REPJ = ct("REPJ", REPJc)
    RH4 = ct("RH4", RH4c)
    II64b = ct("II64b", bf16(II64c))
    NII64b = ct("NII64b", bf16(-II64c))
    NMKb = ct("NMKb", bf16(NMKc))

    kvw = nc.dram_tensor("kvw", [N, ROWU], BF16)

    with tile.TileContext(nc) as tc, contextlib.ExitStack() as ctx:
        singles = ctx.enter_context(tc.tile_pool(name="singles", bufs=1))
        ppdist = ctx.enter_context(tc.tile_pool(name="ppdist", bufs=2, space="PSUM"))
        ppair = ctx.enter_context(tc.tile_pool(name="ppair", bufs=2, space="PSUM"))
        ppe = ctx.enter_context(tc.tile_pool(name="ppe", bufs=2, space="PSUM"))
        psmall = ctx.enter_context(tc.tile_pool(name="psmall", bufs=2, space="PSUM"))
        work = ctx.enter_context(tc.tile_pool(name="work", bufs=2))
        workD = ctx.enter_context(tc.tile_pool(name="workD", bufs=3))
        work3 = ctx.enter_context(tc.tile_pool(name="work3", bufs=2))
        small = ctx.enter_context(tc.tile_pool(name="small", bufs=3))

        # ---------- constants / weights ----------
        def load(name, dram, shape, dtype=F32):
            t = singles.tile(shape, dtype, tag=name)
            nc.sync.dma_start(out=t, in_=dram[:, :])
            return t

        s_WvT = load("WvT", WvT65, [CIN + 1, COUT])
        s_Wk = load("Wk", Wk, [COUT, CIN])
        s_bkc = load("bkc", bkc, [COUT, 1])
        s_Wp2 = load("Wp2", Wp2, [COUT, COUT])
        s_WfcT = load("WfcT", WfcT, [COUT, COUT])
        s_bp2c = load("bp2c", bp2c, [COUT, 1])
        s_bfc = load("bfc", bfc, [1, COUT])
        s_S16 = load("S16", S16, [COUT, H])
        s_I128 = load("I128", I128, [128, 128])
        s_REPJ = load("REPJ", REPJ, [16, 128])
        s_RH4 = load("RH4", RH4, [H, COUT])
        s_II64b = load("II64b", II64b, [128, COUT], BF16)
        s_NII64b = load("NII64b", NII64b, [128, COUT], BF16)
        s_NMKb = load("NMKb", NMKb, [128, H], BF16)

        s_Wp1T = singles.tile([4, COUT], F32)  # rows 0-2 Wp1T, row 3 bp1
        nc.sync.dma_start(out=s_Wp1T[0:3, :], in_=Wp1T[:, :])
        nc.sync.dma_start(out=s_Wp1T[3:4, :], in_=bp1[:, :])

        # ---------- per-call inputs ----------
        # candidate x^T fp8 [64, N] -> f32 [65, N] (row 64 = ones for bias)
        s_x8 = singles.tile([CIN, N], F8E4, tag="x8")
        nc.sync.dma_start(out=s_x8, in_=x8T[:, :])
        s_xT = singles.tile([CIN + 1, N], F32, tag="xT")
        nc.scalar.activation(out=s_xT[0:CIN, :], in_=s_x8, func=AF.Copy)
        nc.vector.memset(s_xT[CIN : CIN + 1, :], 1.0)

        # query pos [4, Q]: rows 0-2 own pos half, row 3 ones.
        # memset whole tile first — DVE writes must start at partition 0.
        s_pq = singles.tile([4, Q], F32, tag="pq")
        nc.vector.memset(s_pq, 1.0)
        nc.sync.dma_start(out=s_pq[0:3, :], in_=pqT[:, :])

        # Wp2T68: cols 0-63 = Wp2T, cols 64-67 = Wp2R (head-mean of Wp2)
        s_Wp2T68 = singles.tile([COUT, COUT + H], F32)
        nc.sync.dma_start(out=s_Wp2T68[:, 0:COUT], in_=Wp2T[:, :])
        p_wp2r = ppair.tile([COUT, H], F32, tag="pair")
        nc.tensor.matmul(out=p_wp2r, lhsT=s_Wp2, rhs=s_S16, start=True, stop=True)
        nc.scalar.activation(out=s_Wp2T68[:, COUT:], in_=p_wp2r, func=AF.Copy)

        # WkR65 [65, 64]: cols 0-3 = head-mean of Wk (+ bk mean in row 64)
        s_WkR = singles.tile([CIN + 1, COUT], F32)
        nc.vector.memset(s_WkR, 0)
        p_wkr = ppair.tile([CIN, H], F32, tag="pair")
        nc.tensor.matmul(out=p_wkr, lhsT=s_Wk, rhs=s_S16, start=True, stop=True)
        nc.scalar.activation(out=s_WkR[0:CIN, 0:H], in_=p_wkr, func=AF.Copy)
        p_bkr = ppair.tile([1, H], F32, tag="pair")
        nc.tensor.matmul(out=p_bkr, lhsT=s_bkc, rhs=s_S16, start=True, stop=True)
        nc.scalar.activation(out=s_WkR[CIN : CIN + 1, 0:H], in_=p_bkr, func=AF.Copy)

        # bias_out [1, 64] = bp2 @ WfcT + bfc
        s_biaso = singles.tile([1, COUT], F32)
        p_bo = ppair.tile([1, COUT], F32, tag="pair")
        nc.tensor.matmul(out=p_bo, lhsT=s_bp2c, rhs=s_WfcT, start=True, stop=True)
        nc.vector.tensor_tensor(s_biaso, p_bo, s_bfc, OP.add)

        # kxn_pos [4, N]: rows 0-2 = candidate posT, row 3 = -0.5 * ||pos_n||^2
        s_kxn = singles.tile([4, N], F32)
        nc.sync.dma_start(out=s_kxn[0:3, :], in_=pnT[:, :])
        s_sq_full = workD.tile([128, N], F32, tag="s_dist")
        s_sq = s_sq_full[0:3, :]
        nc.scalar.activation(out=s_sq, in_=s_kxn[0:3, :], func=AF.Square)
        s_ones3 = singles.tile([3, 1], F32)
        nc.vector.memset(s_ones3, 1.0)
        s_ones1 = singles.tile([1, 128], F32)
        nc.vector.memset(s_ones1, 1.0)
        s_xx = singles.tile([1, N], F32)
        for c in range(NCHUNK):
            p_xx = ppair.tile([1, 512], F32, tag="pair")
            nc.tensor.matmul(
                out=p_xx, lhsT=s_ones3, rhs=s_sq[:, c * 512 : (c + 1) * 512],
                start=True, stop=True,
            )
            nc.scalar.activation(
                out=s_xx[:, c * 512 : (c + 1) * 512], in_=p_xx,
                func=AF.Copy, scale=-0.5,
            )
        nc.sync.dma_start(out=s_kxn[3:4, :], in_=s_xx)

        # ---------- gather source rows kvw [N, 384] bf16 ----------
        for c in range(32):
            csl = slice(c * 128, (c + 1) * 128)
            p_row = ppair.tile([128, 192], F32, tag="pair")
            nc.tensor.matmul(
                out=p_row[:, 0:COUT], lhsT=s_xT[:, csl], rhs=s_WvT,
                start=True, stop=True,
            )
            nc.tensor.matmul(
                out=p_row[:, COUT : 2 * COUT], lhsT=s_kxn[0:3, csl],
                rhs=s_Wp1T[0:3, :], start=True, stop=True,
            )
            nc.tensor.matmul(
                out=p_row[:, 2 * COUT :], lhsT=s_xT[:, csl], rhs=s_WkR,
                start=True, stop=True,
            )
            stg = work.tile([128, ROWU], BF16, tag="stg")
            src3 = _ap(p_row[:, 0:192], [[64, 3], [1, 64]])
            hi3 = _ap(stg[:, 0:ROWU], [[128, 3], [1, 64]])
            lo3 = _ap(stg[:, 64:ROWU], [[128, 3], [1, 64]])
            nc.scalar.activation(out=hi3, in_=src3, func=AF.Copy)
            nc.vector.scalar_tensor_tensor(
                out=lo3, in0=src3, scalar=1.0, in1=hi3, op0=OP.mult,
                op1=OP.subtract,
            )
            nc.sync.dma_start(out=kvw[csl, :], in_=stg)

        # ---------- per q-tile pipeline (2-deep software pipeline) ----------
        def emit_dist(t):
            qsl = slice(t * 128, (t + 1) * 128)
            s_dist = workD.tile([128, N], F32, tag="s_dist")
            for dc in range(NCHUNK):
                p_dist = ppdist.tile([128, 512], F32, tag="p_dist")
                nc.tensor.matmul(
                    out=p_dist,
                    lhsT=s_pq[:, qsl],
                    rhs=s_kxn[:, dc * 512 : (dc + 1) * 512],
                    start=True, stop=True,
                )
                nc.scalar.activation(
                    out=s_dist[:, dc * 512 : (dc + 1) * 512], in_=p_dist,
                    func=AF.Copy,
                )
            return s_dist

        def emit_body(t, s_dist):
            """topk + gather + pair MLP + column-space softmax/aggregation."""
            v8a = small.tile([128, 8], F32, tag="v8a")
            v8b = small.tile([128, 8], F32, tag="v8b")
            idx16 = small.tile([128, K], U32, tag="idx16")
            nc.vector.max(out=v8a, in_=s_dist)
            nc.vector.max_index(out=idx16[:, 0:8], in_max=v8a, in_values=s_dist)
            nc.vector.match_replace(
                out=s_dist, in_to_replace=v8a, in_values=s_dist, imm_value=NEG_BIG
            )
            nc.vector.max(out=v8b, in_=s_dist)
            nc.vector.max_index(out=idx16[:, 8:16], in_max=v8b, in_values=s_dist)

            idxf = small.tile([128, K], F32, tag="idxf")
            nc.vector.tensor_copy(idxf, idx16)
            p_idxT = psmall.tile([K, 128], F32, tag="sm")
            nc.tensor.transpose(out=p_idxT, in_=idxf, identity=s_I128)
            s_idxT = small.tile([K, 128], F32, tag="s_idxT")
            nc.vector.tensor_copy(s_idxT, p_idxT)
            p_idxrep = psmall.tile([128, 128], F32, tag="sm")
            nc.tensor.matmul(
                out=p_idxrep, lhsT=s_REPJ, rhs=s_idxT, start=True, stop=True
            )
            idxs16 = small.tile([128, 128], I16, tag="idxs16")
            nc.vector.tensor_copy(idxs16, p_idxrep)

            s_aggT = work3.tile([COUT, 128], F32, tag="s_aggT")
            for c in range(4):
                q0 = t * 128 + c * 32
                csl = slice(c * 32, (c + 1) * 32)
                g = work.tile([128, 3, 512], BF16, tag="g")
                nc.gpsimd.dma_gather(
                    out_ap=g, in_ap=kvw[:, :],
                    idxs_ap=idxs16[:, csl],
                    num_idxs=512, num_idxs_reg=512, elem_size=ROWU,
                    transpose=True,
                )
                p_P = ppair.tile([COUT, 512], F32, tag="pair")
                posrep = _ap(s_pq[:, q0 : q0 + 32], [[1, 32], [0, 16]])
                nc.tensor.matmul(
                    out=p_P, lhsT=s_Wp1T, rhs=posrep, start=True, stop=False
                )
                nc.tensor.matmul(
                    out=p_P, lhsT=s_NII64b, rhs=g[:, 1, :],
                    start=False, stop=True,
                )
                s_relu = work.tile([COUT, 512], F32, tag="s_relu")
                nc.scalar.activation(out=s_relu, in_=p_P, func=AF.Relu)

                p_pe = ppe.tile([COUT + H, 512], F32, tag="pe")
                nc.tensor.matmul(
                    out=p_pe[0:COUT, :], lhsT=s_Wp2T68[:, 0:COUT], rhs=s_relu,
                    start=True, stop=False,
                )
                nc.tensor.matmul(
                    out=p_pe[0:COUT, :], lhsT=s_II64b, rhs=g[:, 0, :],
                    start=False, stop=True,
                )
                nc.tensor.matmul(
                    out=p_pe[COUT:, :], lhsT=s_Wp2T68[:, COUT:], rhs=s_relu,
                    start=True, stop=False,
                )
                nc.tensor.matmul(
                    out=p_pe[COUT:, :], lhsT=s_NMKb, rhs=g[:, 2, :],
                    start=False, stop=True,
                )
                # softmax over j, normalized in (q,j)-column space
                s_exp = small.tile([H, 512], F32, tag="s_exp")
                nc.scalar.activation(out=s_exp, in_=p_pe[COUT:, :], func=AF.Exp)
                s_sig = small.tile([H, 32], F32, tag="s_sig")
                nc.vector.tensor_reduce(
                    out=s_sig, in_=_ap(s_exp[:, :], [[16, 32], [1, 16]]),
                    axis=AX.X, op=OP.add,
                )
                s_rec = small.tile([H, 32], F32, tag="s_rec")
                nc.vector.reciprocal(s_rec, s_sig)
                s_attn = small.tile([H, 512], F32, tag="s_attn")
                nc.vector.tensor_tensor(
                    s_attn, s_exp, _ap(s_rec[:, :], [[1, 32], [0, 16]]), OP.mult
                )
                # replicate attn to the 16 per-head features and aggregate
                p_a64 = ppair.tile([COUT, 512], F32, tag="pair")
                nc.tensor.matmul(
                    out=p_a64, lhsT=s_RH4, rhs=s_attn, start=True, stop=True
                )
                s_vpe = work.tile([COUT, 512], F32, tag="s_vpe")
                nc.scalar.activation(out=s_vpe, in_=p_pe[0:COUT, :], func=AF.Copy)
                s_wv = work.tile([COUT, 512], F32, tag="s_wv")
                nc.vector.tensor_tensor(s_wv, s_vpe, p_a64, OP.mult)
                nc.vector.tensor_reduce(
                    out=s_aggT[:, csl], in_=_ap(s_wv[:, :], [[16, 32], [1, 16]]),
                    axis=AX.X, op=OP.add,
                )

            qsl = slice(t * 128, (t + 1) * 128)
            p_out = psmall.tile([COUT, 128], F32, tag="sm")
            nc.tensor.matmul(out=p_out, lhsT=s_WfcT, rhs=s_aggT, start=True, stop=False)
            nc.tensor.matmul(
                out=p_out, lhsT=s_biaso, rhs=s_ones1,
                start=False, stop=True,
            )
            s_out = small.tile([COUT, 128], F8E4, tag="s_out")
            nc.scalar.activation(out=s_out, in_=p_out, func=AF.Copy)
            nc.sync.dma_start(out=dT[:, qsl], in_=s_out)

        s_dist_next = emit_dist(0)
        for t in range(NT):
            s_dist = s_dist_next
            if t + 1 < NT:
                s_dist_next = emit_dist(t + 1)
            emit_body(t, s_dist)

    # nc.m is frozen once compile() returns, but the jit lowering re-serializes
    # the 3MB BIR JSON on every call (~20ms).  Freeze the serialization on this
    # instance right after compile.
    orig_compile = nc.compile

    def _compile_and_freeze(*a, **kw):
        r = orig_compile(*a, **kw)
        blob = bass.Bass.to_json_bytes(nc)
        nc.to_json_bytes = lambda: blob
        return r

    nc.compile = _compile_and_freeze
    return nc


def make_in_maps(inputs):
    import ml_dtypes

    x = np.asarray(inputs["x"], np.float32)
    pos = np.asarray(inputs["pos"], np.float32)

    in_maps = []
    for core in range(8):
        b, qh = core // 2, core % 2
        qs = slice(qh * Q, (qh + 1) * Q)
        in_maps.append({
            "x8T": np.ascontiguousarray(x[b].T).astype(ml_dtypes.float8_e4m3),
            "pqT": np.ascontiguousarray(pos[b, qs].T),
            "pnT": np.ascontiguousarray(pos[b].T),
        })
    return in_maps


def kernel(**inputs):
    from concourse.bass_utils import run_bass_kernel_spmd

    nc = build_nc(inputs)
    nc.compile()
    in_maps = make_in_maps(inputs)
    res = run_bass_kernel_spmd(nc, in_maps, list(range(8)))
    x = np.asarray(inputs["x"], np.float32)
    out = np.empty((B, N, COUT), np.float32)
    for core in range(8):
        b, qh = core // 2, core % 2
        qs = slice(qh * Q, (qh + 1) * Q)
        delta = np.asarray(res.results[core]["dT"], np.float32).T
        out[b, qs, :] = x[b, qs, :] + delta
    return out
